# revision 7
# baseline (speedup 1.0000x reference)
"""Alpha-filter (keras_spiking AlphaCell) Trainium2 Bass kernel — matmul scan.

Math: per (batch b, feature k) the reference runs the 2-state recurrence
    x_t = A_k x_{t-1} + B_k u_t,   y_t = x_t[1]
which reduces (see kernel_baseline.py) to the causal convolution
    y_t = sum_{j<=t} h_{t-j} u_j + carry,   h_m = cS e^m + cEta m e^{m-1}.

Each 128-step time block is computed with PE matmuls via the separable
factorization (input rows i, output rows r, features k on the free dim):
    h_{r-i} = e^r e^{-i} [ cS + cEta' (r-i) ],   cEta' = cEta/e
    v1 = (cS e^{-i}) (.) u,  v2 = (cEta' e^{-i}) (.) u     (prescales)
    P  = Tril^T v1 + TrilM^T v2 + Wr^T R        (bf16 matmuls, one PSUM bank)
    y  = E (.) P                                (elementwise combine)
with Tril[i,r] = [i<=r], TrilM[i,r] = (r-i)[i<=r] (the (r-i) factor lives
entirely in the weights — no large-coefficient cancellation, so bf16
inputs cost only ~4e-3 equivalent input noise), E[r,k] = e^r, and R the
2-row carry inject (rows r1 = e cEta ce + (e cS + cEta) cs, r2 = cEta cs;
y += E (r1 + r r2)).

Cross-block carries (cs = s-state, ce = eta-state entering each block):
per-block summary matmuls accumulate Sm0 = sum_i v2, Sm1 = sum_i i v2
into a [16,512] PSUM tile per batch; batches are processed in two halves
of 4, and per half ONE transposed layout [128k, (16 bc)(2 s)(8 m)]
(bc = 4*b_loc + c) feeds a fully batched carry chain:
    cs' = e^L cs + k1 Sm0,   ce' = e^L ce + bL cs + k2 Sm0 - k3 Sm1
done with ~14 wide DVE ops + 2 reset-trick scans (data0 = 0 at every
m = 0 kills the carry across (b, c) boundaries, so one scan instruction
covers all 16 independent 8-step recurrences).  The resulting R rows are
transposed back per batch for the inject matmuls.

Engines: GpSimd does v1 prescales, DVE does v2 + chain + pair-merged
combines ([128,1024] over a 2-bank PSUM pair tile), PE does matmuls,
Act does PSUM copies and out-DMA issue.  No data transposes anywhere —
inputs stay in their natural [time, feature] layout.

Sharding: data-parallel over batch, 8 batches per core x 8 cores.
"""

import sys

for _p in ("/opt/trn_rl_repo",):
    if _p not in sys.path:
        sys.path.insert(0, _p)

from contextlib import ExitStack

import ml_dtypes
import numpy as np

import concourse.bacc as bacc
import concourse.bass as bass
import concourse.tile as tile
from concourse import mybir
from concourse.bass_utils import run_bass_kernel_spmd

DT = 0.001
B, T, K = 64, 1024, 512
N_CORES = 8
B_LOC = B // N_CORES  # 8 batches per core
P = 128
NBLK = T // P   # 8 time blocks of 128
KC = K // P     # 4 feature chunks of 128 (carry chain only)
HB = 4          # batches per chain half
NBC = HB * KC   # 16 (b_loc, c) pairs per half

F32 = mybir.dt.float32
BF16 = mybir.dt.bfloat16
MULT = mybir.AluOpType.mult
ADD = mybir.AluOpType.add
SUB = mybir.AluOpType.subtract


def _ap(base, off_elems, dims):
    """Custom AP: base tile AP -> new free dims (keeps partition dim)."""
    return bass.AP(tensor=base.tensor, offset=base.offset + off_elems,
                   ap=[base.ap[0]] + dims)


def build_nc():
    nc = bacc.Bacc(None, target_bir_lowering=False)

    x = nc.dram_tensor("x", [B_LOC, T, K], F32, kind="ExternalInput")
    gte = nc.dram_tensor("gte", [P, 4 * K], F32, kind="ExternalInput")
    wmat = nc.dram_tensor("wmat", [P, 3 * P], BF16, kind="ExternalInput")
    wr = nc.dram_tensor("wr", [16, NBLK * P], BF16, kind="ExternalInput")
    ident = nc.dram_tensor("ident", [P, P], F32, kind="ExternalInput")
    chc = nc.dram_tensor("chc", [P, 11 * NBC + NBC * NBLK], F32,
                         kind="ExternalInput")
    y = nc.dram_tensor("y", [B_LOC, T, K], F32, kind="ExternalOutput")

    with tile.TileContext(nc) as tc, ExitStack() as ctx:
        singles = ctx.enter_context(tc.tile_pool(name="singles", bufs=1))
        inpool = ctx.enter_context(tc.tile_pool(name="inpool", bufs=3))
        vpool = ctx.enter_context(tc.tile_pool(name="vpool", bufs=4))
        smpool = ctx.enter_context(tc.tile_pool(name="smpool", bufs=2))
        chpool = ctx.enter_context(tc.tile_pool(name="chpool", bufs=2))
        rpool = ctx.enter_context(tc.tile_pool(name="rpool", bufs=4))
        outpool = ctx.enter_context(tc.tile_pool(name="outpool", bufs=2))
        ppool = ctx.enter_context(tc.tile_pool(name="ppool", bufs=2, space="PSUM"))
        smps = ctx.enter_context(tc.tile_pool(name="smps", bufs=1, space="PSUM"))
        stps = ctx.enter_context(tc.tile_pool(name="stps", bufs=2, space="PSUM"))
        rps = ctx.enter_context(tc.tile_pool(name="rps", bufs=1, space="PSUM"))

        # ---- one-time constant loads -----------------------------------
        gte_t = singles.tile([P, 4 * K], F32)
        nc.sync.dma_start(out=gte_t[:], in_=gte[:])
        wmat_t = singles.tile([P, 3 * P], BF16)
        nc.sync.dma_start(out=wmat_t[:], in_=wmat[:])
        wr_t = singles.tile([16, NBLK * P], BF16)
        nc.sync.dma_start(out=wr_t[:], in_=wr[:])
        ident_t = singles.tile([P, P], F32)
        nc.sync.dma_start(out=ident_t[:], in_=ident[:])
        chc_t = singles.tile([P, 11 * NBC + NBC * NBLK], F32)
        nc.sync.dma_start(out=chc_t[:], in_=chc[:])

        # PE warm-up during the initial DMA window (HAM clock ramp).
        scratch = singles.tile([P, P], F32)
        nc.gpsimd.memset(scratch[:], 0.0)
        warm = ppool.tile([P, 2, K], F32, name="warm", tag="pt")
        for _ in range(6):
            nc.tensor.matmul(warm[:, 0, 0:P], scratch[:], scratch[:],
                             start=True, stop=True)

        chb = chc_t[:]

        def cc16(j):
            # const block j as [128, 16] slice
            return chc_t[:, j * NBC:(j + 1) * NBC]

        def cbc(j, n=NBLK):
            # const block j broadcast over m: [128, 16, n]
            return _ap(chb, j * NBC, [[1, NBC], [0, n]])

        D0OFF = 11 * NBC
        d0_2d = chc_t[:, D0OFF:D0OFF + NBC * NBLK]

        g1b = _ap(gte_t[:], 0, [[0, 4], [1, K]])
        g2b = _ap(gte_t[:], K, [[0, 4], [1, K]])
        e2_tile = gte_t[:, 2 * K: 4 * K]

        w_tril = wmat_t[:, 0:P]
        w_trilM = wmat_t[:, P: 2 * P]

        vs = {}
        rsb = {}
        sts = {}

        def emit_A(b):
            """DMA-in, prescales, summary matmuls, SmAll copy, ST transpose."""
            if b % HB == 0:
                sts[b // HB] = stps.tile([P, NBC * 16], F32, tag="st",
                                         name=f"st{b // HB}")
            st_ps = sts[b // HB]
            b_loc = b % HB
            in_stage = inpool.tile([P, NBLK, K], F32)
            xv = x[b].rearrange("(a p) k -> p a k", p=P)
            v1 = vpool.tile([P, NBLK, K], BF16, tag="v1", name=f"v1_{b}")
            v2 = vpool.tile([P, NBLK, K], BF16, tag="v2", name=f"v2_{b}")
            sm_ps = smps.tile([16, K], F32)
            for h in (0, 4):
                nc.sync.dma_start(out=in_stage[:, h:h + 4, :],
                                  in_=xv[:, h:h + 4, :])
                nc.gpsimd.tensor_tensor(out=v1[:, h:h + 4, :],
                                        in0=in_stage[:, h:h + 4, :],
                                        in1=g1b, op=MULT)
                nc.vector.tensor_tensor(out=v2[:, h:h + 4, :],
                                        in0=in_stage[:, h:h + 4, :],
                                        in1=g2b, op=MULT)
                for m in range(h, h + 4):
                    nc.tensor.matmul(
                        sm_ps[:],
                        wmat_t[:, 2 * P + 16 * m: 2 * P + 16 * (m + 1)],
                        v2[:, m, :],
                        start=(m == 0), stop=(m == NBLK - 1))
            sm_sb = smpool.tile([16, K], F32)
            nc.scalar.copy(sm_sb[:], sm_ps[:])
            for c in range(KC):
                bc = b_loc * KC + c
                nc.tensor.matmul(st_ps[:, bc * 16:(bc + 1) * 16],
                                 sm_sb[:, c * P:(c + 1) * P],
                                 ident_t[0:16, 0:16], is_transpose=True,
                                 skip_group_check=True)
            vs[b] = (v1, v2)

        def emit_H(half):
            """Batched carry chain for 4 batches -> R rows per batch."""
            st_ps = sts[half]
            stb = st_ps[:]
            sm0 = _ap(stb, 0, [[16, NBC], [1, NBLK]])
            sm1 = _ap(stb, 8, [[16, NBC], [1, NBLK]])

            in1 = chpool.tile([P, NBC * NBLK], F32, tag="in1")
            in2 = chpool.tile([P, NBC * NBLK], F32, tag="in2")
            t1 = chpool.tile([P, NBC * NBLK], F32, tag="t1")
            d2 = chpool.tile([P, NBC * NBLK], F32, tag="d2")
            csp = chpool.tile([P, NBC * NBLK], F32, tag="csp")
            cep = chpool.tile([P, NBC * NBLK], F32, tag="cep")
            rt = chpool.tile([P, NBC, 2, NBLK], F32, tag="rt")
            in1b, in2b, t1b, d2b = in1[:], in2[:], t1[:], d2[:]
            cs_prev = _ap(csp[:], 0, [[NBLK, NBC], [1, NBLK]])
            ce_prev = _ap(cep[:], 0, [[NBLK, NBC], [1, NBLK]])
            # shifted views: entries for scan position m hold inputs of step
            # m-1; position 0 holds the initial carry (reset trick: d0=0 there)
            sm0s = _ap(stb, 0, [[16, NBC], [1, NBLK - 1]])
            sm1s = _ap(stb, 8, [[16, NBC], [1, NBLK - 1]])

            def shifted(b_):
                return _ap(b_, 1, [[NBLK, NBC], [1, NBLK - 1]])

            def col0(b_):
                return _ap(b_, 0, [[NBLK, NBC]])

            nc.vector.tensor_tensor(out=shifted(in1b), in0=sm0s,
                                    in1=cbc(0, NBLK - 1), op=MULT)
            nc.scalar.copy(col0(in1b), cc16(9))
            nc.vector.tensor_tensor(out=shifted(in2b), in0=sm0s,
                                    in1=cbc(1, NBLK - 1), op=MULT)
            nc.vector.tensor_tensor(out=shifted(t1b), in0=sm1s,
                                    in1=cbc(2, NBLK - 1), op=MULT)
            nc.vector.tensor_tensor(out=shifted(in2b), in0=shifted(in2b),
                                    in1=shifted(t1b), op=SUB)
            nc.vector.tensor_tensor_scan(
                out=csp[:], data0=d0_2d, data1=in1[:],
                initial=0.0, op0=MULT, op1=ADD)
            nc.vector.tensor_tensor(
                out=shifted(d2b),
                in0=_ap(csp[:], 0, [[NBLK, NBC], [1, NBLK - 1]]),
                in1=cbc(3, NBLK - 1), op=MULT)
            nc.vector.tensor_tensor(out=shifted(d2b), in0=shifted(d2b),
                                    in1=shifted(in2b), op=ADD)
            nc.scalar.copy(col0(d2b), cc16(10))
            nc.vector.tensor_tensor_scan(
                out=cep[:], data0=d0_2d, data1=d2[:],
                initial=0.0, op0=MULT, op1=ADD)
            # r1 = mu1*ce_prev + mu2*cs_prev ; r2 = nu*cs_prev
            nc.vector.tensor_tensor(out=t1[:], in0=ce_prev, in1=cbc(4), op=MULT)
            nc.vector.tensor_tensor(out=in1[:], in0=cs_prev, in1=cbc(5), op=MULT)
            nc.vector.tensor_tensor(out=_ap(rt[:], 0, [[2 * NBLK, NBC], [1, NBLK]]),
                                    in0=t1[:], in1=in1[:], op=ADD)
            nc.vector.tensor_tensor(out=_ap(rt[:], NBLK, [[2 * NBLK, NBC], [1, NBLK]]),
                                    in0=cs_prev, in1=cbc(6), op=MULT)

            for b_loc in range(HB):
                b = half * HB + b_loc
                r_ps = rps.tile([16, K], F32, tag="rps", name=f"rps{b}")
                for c in range(KC):
                    bc = b_loc * KC + c
                    nc.tensor.matmul(r_ps[:, c * P:(c + 1) * P],
                                     rt[:, bc, :, :], ident_t[:],
                                     is_transpose=True, skip_group_check=True)
                r_sb = rpool.tile([16, K], BF16, tag="rsb", name=f"rsb{b}")
                nc.scalar.copy(r_sb[:], r_ps[:])
                rsb[b] = r_sb

        def emit_C(b):
            """Main block matmuls + pair-merged combine + DMA out."""
            v1, v2 = vs.pop(b)
            r_sb = rsb.pop(b)
            out_stage = outpool.tile([P, NBLK, K], F32)
            yv = y[b].rearrange("(a p) k -> p a k", p=P)
            for j in range(NBLK // 2):
                pt = ppool.tile([P, 2, K], F32, tag="pt", name=f"pt{b}_{j}")
                for half_i in (0, 1):
                    m = 2 * j + half_i
                    nc.tensor.matmul(pt[:, half_i, :], w_tril, v1[:, m, :],
                                     start=True, stop=False,
                                     skip_group_check=True)
                    nc.tensor.matmul(pt[:, half_i, :], w_trilM, v2[:, m, :],
                                     start=False, stop=False,
                                     skip_group_check=True)
                    nc.tensor.matmul(pt[:, half_i, :],
                                     wr_t[:, m * P:(m + 1) * P], r_sb[:],
                                     start=False, stop=True,
                                     skip_group_check=True)
                nc.vector.tensor_tensor(out=out_stage[:, 2 * j:2 * j + 2, :],
                                        in0=pt[:], in1=e2_tile, op=MULT)
                nc.scalar.dma_start(out=yv[:, 2 * j:2 * j + 2, :],
                                    in_=out_stage[:, 2 * j:2 * j + 2, :])

        # ---- software-pipelined emission -------------------------------
        # A0..A3 H0 C0 A4 C1 A5 C2 A6 C3 A7 H1 C4..C7
        for b in range(HB):
            emit_A(b)
        emit_H(0)
        emit_C(0)
        for b in range(1, HB):
            emit_A(HB + b - 1)
            emit_C(b)
        emit_A(2 * HB - 1)
        emit_H(1)
        for b in range(HB, 2 * HB):
            emit_C(b)

    nc.compile()
    return nc


_CACHE = {}
PROFILE = False
LAST_RESULT = None


def _host_constants(initial_level, tau):
    tau_c = np.maximum(tau.astype(np.float64), 1e-8)
    a = DT / tau_c
    e = np.exp(-a)
    em1 = 1.0 - e
    cEta = e * a * em1
    cS = em1 - e * a
    cEtp = a * em1  # cEta / e
    lvl = initial_level.astype(np.float64)

    i = np.arange(P, dtype=np.float64)[:, None]
    r = np.arange(P, dtype=np.float64)
    einv = np.exp(a[None, :] * i)               # e^{-i}
    G1 = cS[None, :] * einv
    G2 = cEtp[None, :] * einv
    E = np.exp(-a[None, :] * i)
    gte = np.concatenate([G1, G2, E, E], axis=1).astype(np.float32)

    w_tril = (i <= r[None, :]).astype(np.float64)
    w_trilM = w_tril * (r[None, :] - i)
    wsum = np.zeros((P, P), dtype=np.float64)
    for m in range(NBLK):
        wsum[:, 16 * m + m] = 1.0
        wsum[:, 16 * m + 8 + m] = i[:, 0]
    wmat = np.concatenate([w_tril, w_trilM, wsum], axis=1).astype(
        ml_dtypes.bfloat16)

    wr_ = np.zeros((16, NBLK * P), dtype=np.float64)
    for m in range(NBLK):
        wr_[m, m * P:(m + 1) * P] = 1.0
        wr_[8 + m, m * P:(m + 1) * P] = r
    wr_ = wr_.astype(ml_dtypes.bfloat16)

    eL = e ** 128
    e127 = e ** 127
    e126 = e ** 126
    cs0 = lvl / em1
    ce0 = lvl / (em1 * em1)
    consts = [
        e127 / cEtp,           # 0 k1
        127.0 * e126 / cEtp,   # 1 k2
        e126 / cEtp,           # 2 k3
        128.0 * e127,          # 3 bL
        e * cEta,              # 4 mu1
        e * cS + cEta,         # 5 mu2
        cEta,                  # 6 nu
        eL * cs0,              # 7 alpha_e*cs0  (m=0 fixup)
        eL * ce0,              # 8 alpha_e*ce0  (m=0 fixup)
        cs0,                   # 9
        ce0,                   # 10
    ]
    chc = np.zeros((P, 11 * NBC + NBC * NBLK), dtype=np.float64)
    for bc in range(NBC):
        c = bc % KC
        sl = slice(c * P, (c + 1) * P)
        for j, v in enumerate(consts):
            chc[:, j * NBC + bc] = v[sl]
        # d0: scan multiplier alpha_e, 0 at m=0 (reset)
        for m in range(NBLK):
            chc[:, 11 * NBC + bc * NBLK + m] = 0.0 if m == 0 else eL[sl]
    chc = chc.astype(np.float32)

    ident = np.eye(P, dtype=np.float32)
    return gte, wmat, wr_, ident, chc


def kernel(inputs, initial_level, tau):
    global LAST_RESULT
    inputs = np.ascontiguousarray(np.asarray(inputs, dtype=np.float32))
    initial_level = np.asarray(initial_level, dtype=np.float32)
    tau = np.asarray(tau, dtype=np.float32)
    assert inputs.shape == (B, T, K), inputs.shape

    gte, wmat, wr_, ident, chc = _host_constants(initial_level, tau)

    if "nc" not in _CACHE:
        _CACHE["nc"] = build_nc()
    nc = _CACHE["nc"]

    in_maps = [
        {
            "x": inputs[i * B_LOC: (i + 1) * B_LOC],
            "gte": gte,
            "wmat": wmat,
            "wr": wr_,
            "ident": ident,
            "chc": chc,
        }
        for i in range(N_CORES)
    ]
    res = run_bass_kernel_spmd(nc, in_maps, list(range(N_CORES)), trace=PROFILE)
    LAST_RESULT = res
    return np.concatenate([r["y"] for r in res.results], axis=0)


# revision 8
# speedup vs baseline: 1.2456x; 1.2456x over previous
"""Alpha-filter (keras_spiking AlphaCell) Trainium2 Bass kernel — matmul scan.

Math: per (batch b, feature k) the reference runs the 2-state recurrence
    x_t = A_k x_{t-1} + B_k u_t,   y_t = x_t[1]
which reduces (see kernel_baseline.py) to the causal convolution
    y_t = sum_{j<=t} h_{t-j} u_j + carry,   h_m = cS e^m + cEta m e^{m-1}.

Each 128-step time block is computed with PE matmuls via the separable
factorization (input rows i, output rows r, features k on the free dim):
    h_{r-i} = e^r e^{-i} [ cS + cEta' (r-i) ],   cEta' = cEta/e
    v1 = (cS e^{-i}) (.) u,  v2 = (cEta' e^{-i}) (.) u     (prescales)
    P  = Tril^T v1 + TrilM^T v2 + Wr^T R        (bf16 matmuls, one PSUM bank)
    y  = E (.) P                                (elementwise combine)
with Tril[i,r] = [i<=r], TrilM[i,r] = (r-i)[i<=r] (the (r-i) factor lives
entirely in the weights — no large-coefficient cancellation, so bf16
inputs cost only ~4e-3 equivalent input noise), E[r,k] = e^r, and R the
2-row carry inject (rows r1 = e cEta ce + (e cS + cEta) cs, r2 = cEta cs;
y += E (r1 + r r2)).

Cross-block carries (cs = s-state, ce = eta-state entering each block):
per-block summary matmuls accumulate Sm0 = sum_i v2, Sm1 = sum_i i v2
into a [16,512] PSUM tile per batch; batches are processed in two halves
of 4, and per half ONE transposed layout [128k, (16 bc)(2 s)(8 m)]
(bc = 4*b_loc + c) feeds a fully batched carry chain:
    cs' = e^L cs + k1 Sm0,   ce' = e^L ce + bL cs + k2 Sm0 - k3 Sm1
done with ~14 wide DVE ops + 2 reset-trick scans (data0 = 0 at every
m = 0 kills the carry across (b, c) boundaries, so one scan instruction
covers all 16 independent 8-step recurrences).  The resulting R rows are
transposed back per batch for the inject matmuls.

Engines: GpSimd does v1 prescales, DVE does v2 + chain + pair-merged
combines ([128,1024] over a 2-bank PSUM pair tile), PE does matmuls,
Act does PSUM copies and out-DMA issue.  No data transposes anywhere —
inputs stay in their natural [time, feature] layout.

Sharding: data-parallel over batch, 8 batches per core x 8 cores.
"""

import sys

for _p in ("/opt/trn_rl_repo",):
    if _p not in sys.path:
        sys.path.insert(0, _p)

from contextlib import ExitStack

import ml_dtypes
import numpy as np

import concourse.bacc as bacc
import concourse.bass as bass
import concourse.tile as tile
from concourse import mybir
from concourse.bass_utils import run_bass_kernel_spmd

DT = 0.001
B, T, K = 64, 1024, 512
N_CORES = 8
B_LOC = B // N_CORES  # 8 batches per core
P = 128
NBLK = T // P   # 8 time blocks of 128
KC = K // P     # 4 feature chunks of 128 (carry chain only)
HB = 4          # batches per chain half
NBC = HB * KC   # 16 (b_loc, c) pairs per half

F32 = mybir.dt.float32
BF16 = mybir.dt.bfloat16
MULT = mybir.AluOpType.mult
ADD = mybir.AluOpType.add
SUB = mybir.AluOpType.subtract


def _ap(base, off_elems, dims):
    """Custom AP: base tile AP -> new free dims (keeps partition dim)."""
    return bass.AP(tensor=base.tensor, offset=base.offset + off_elems,
                   ap=[base.ap[0]] + dims)


def build_nc():
    nc = bacc.Bacc(None, target_bir_lowering=False)

    x = nc.dram_tensor("x", [B_LOC, T, K], F32, kind="ExternalInput")
    gte = nc.dram_tensor("gte", [P, 2 * K], F32, kind="ExternalInput")
    g12 = nc.dram_tensor("g12", [P, 8 * K], F32, kind="ExternalInput")
    wmat = nc.dram_tensor("wmat", [P, 3 * P], BF16, kind="ExternalInput")
    wr = nc.dram_tensor("wr", [16, NBLK * P], BF16, kind="ExternalInput")
    ident = nc.dram_tensor("ident", [P, P], F32, kind="ExternalInput")
    chc = nc.dram_tensor("chc", [P, 11 * NBC + NBC * NBLK], F32,
                         kind="ExternalInput")
    y = nc.dram_tensor("y", [B_LOC, T, K], F32, kind="ExternalOutput")

    with tile.TileContext(nc) as tc, ExitStack() as ctx:
        singles = ctx.enter_context(tc.tile_pool(name="singles", bufs=1))
        inpool = ctx.enter_context(tc.tile_pool(name="inpool", bufs=2))
        vpool = ctx.enter_context(tc.tile_pool(name="vpool", bufs=4))
        smpool = ctx.enter_context(tc.tile_pool(name="smpool", bufs=2))
        chpool = ctx.enter_context(tc.tile_pool(name="chpool", bufs=2))
        rpool = ctx.enter_context(tc.tile_pool(name="rpool", bufs=4))
        outpool = ctx.enter_context(tc.tile_pool(name="outpool", bufs=2))
        stagepool = ctx.enter_context(tc.tile_pool(name="stagepool", bufs=3))
        ppool = ctx.enter_context(tc.tile_pool(name="ppool", bufs=2, space="PSUM"))
        smps = ctx.enter_context(tc.tile_pool(name="smps", bufs=1, space="PSUM"))
        stps = ctx.enter_context(tc.tile_pool(name="stps", bufs=2, space="PSUM"))
        rps = ctx.enter_context(tc.tile_pool(name="rps", bufs=1, space="PSUM"))

        # ---- one-time constant loads -----------------------------------
        gte_t = singles.tile([P, 2 * K], F32)
        nc.sync.dma_start(out=gte_t[:], in_=gte[:])
        g12_t = singles.tile([P, 8 * K], F32)
        nc.sync.dma_start(out=g12_t[:], in_=g12[:])
        wmat_t = singles.tile([P, 3 * P], BF16)
        nc.sync.dma_start(out=wmat_t[:], in_=wmat[:])
        wr_t = singles.tile([16, NBLK * P], BF16)
        nc.sync.dma_start(out=wr_t[:], in_=wr[:])
        ident_t = singles.tile([P, P], F32)
        nc.sync.dma_start(out=ident_t[:], in_=ident[:])
        chc_t = singles.tile([P, 11 * NBC + NBC * NBLK], F32)
        nc.sync.dma_start(out=chc_t[:], in_=chc[:])

        # PE warm-up during the initial DMA window (HAM clock ramp).
        scratch = singles.tile([P, P], F32)
        nc.gpsimd.memset(scratch[:], 0.0)
        warm = ppool.tile([P, 2, K], F32, name="warm", tag="pt")
        for _ in range(6):
            nc.tensor.matmul(warm[:, 0, 0:P], scratch[:], scratch[:],
                             start=True, stop=True)

        chb = chc_t[:]

        def cc16(j):
            # const block j as [128, 16] slice
            return chc_t[:, j * NBC:(j + 1) * NBC]

        def cbc(j, n=NBLK):
            # const block j broadcast over m: [128, 16, n]
            return _ap(chb, j * NBC, [[1, NBC], [0, n]])

        D0OFF = 11 * NBC
        d0_2d = chc_t[:, D0OFF:D0OFF + NBC * NBLK]

        g12b = _ap(g12_t[:], 0, [[4 * K, 2], [1, 4 * K]])
        e2_tile = gte_t[:]

        w_tril = wmat_t[:, 0:P]
        w_trilM = wmat_t[:, P: 2 * P]

        vs = {}
        rsb = {}
        sts = {}

        def emit_A(b):
            """DMA-in, prescales, summary matmuls, SmAll copy, ST transpose."""
            if b % HB == 0:
                sts[b // HB] = stps.tile([P, NBC * 16], F32, tag="st",
                                         name=f"st{b // HB}")
            st_ps = sts[b // HB]
            b_loc = b % HB
            in_stage = inpool.tile([P, NBLK, K], F32)
            xv = x[b].rearrange("(a p) k -> p a k", p=P)
            v12 = vpool.tile([P, 2, NBLK, K], BF16, tag="v12", name=f"v12_{b}")
            v12b = v12[:]
            sm_ps = smps.tile([16, K], F32)
            for h in (0, 4):
                nc.sync.dma_start(out=in_stage[:, h:h + 4, :],
                                  in_=xv[:, h:h + 4, :])
                # both prescales in one op: out[w, m, k] = in[m, k]*G12[w, m, k]
                nc.vector.tensor_tensor(
                    out=_ap(v12b, h * K, [[NBLK * K, 2], [1, 4 * K]]),
                    in0=_ap(in_stage[:], h * K, [[0, 2], [1, 4 * K]]),
                    in1=g12b, op=MULT)
                for m in range(h, h + 4):
                    nc.tensor.matmul(
                        sm_ps[:],
                        wmat_t[:, 2 * P + 16 * m: 2 * P + 16 * (m + 1)],
                        v12[:, 1, m, :],
                        start=(m == 0), stop=(m == NBLK - 1))
            sm_sb = smpool.tile([16, K], F32)
            nc.scalar.copy(sm_sb[:], sm_ps[:])
            for c in range(KC):
                bc = b_loc * KC + c
                nc.tensor.matmul(st_ps[:, bc * 16:(bc + 1) * 16],
                                 sm_sb[:, c * P:(c + 1) * P],
                                 ident_t[0:16, 0:16], is_transpose=True,
                                 skip_group_check=True)
            vs[b] = v12

        def emit_H(half):
            """Batched carry chain for 4 batches -> R rows per batch."""
            st_ps = sts[half]
            stb = st_ps[:]
            sm0 = _ap(stb, 0, [[16, NBC], [1, NBLK]])
            sm1 = _ap(stb, 8, [[16, NBC], [1, NBLK]])

            in1 = chpool.tile([P, NBC * NBLK], F32, tag="in1")
            in2 = chpool.tile([P, NBC * NBLK], F32, tag="in2")
            t1 = chpool.tile([P, NBC * NBLK], F32, tag="t1")
            d2 = chpool.tile([P, NBC * NBLK], F32, tag="d2")
            csp = chpool.tile([P, NBC * NBLK], F32, tag="csp")
            cep = chpool.tile([P, NBC * NBLK], F32, tag="cep")
            rt = chpool.tile([P, NBC, 2, NBLK], F32, tag="rt")
            in1b, in2b, t1b, d2b = in1[:], in2[:], t1[:], d2[:]
            cs_prev = _ap(csp[:], 0, [[NBLK, NBC], [1, NBLK]])
            ce_prev = _ap(cep[:], 0, [[NBLK, NBC], [1, NBLK]])
            # shifted views: entries for scan position m hold inputs of step
            # m-1; position 0 holds the initial carry (reset trick: d0=0 there)
            sm0s = _ap(stb, 0, [[16, NBC], [1, NBLK - 1]])
            sm1s = _ap(stb, 8, [[16, NBC], [1, NBLK - 1]])

            def shifted(b_):
                return _ap(b_, 1, [[NBLK, NBC], [1, NBLK - 1]])

            def col0(b_):
                return _ap(b_, 0, [[NBLK, NBC]])

            nc.vector.tensor_tensor(out=shifted(in1b), in0=sm0s,
                                    in1=cbc(0, NBLK - 1), op=MULT)
            nc.scalar.copy(col0(in1b), cc16(9))
            nc.vector.tensor_tensor(out=shifted(in2b), in0=sm0s,
                                    in1=cbc(1, NBLK - 1), op=MULT)
            nc.vector.tensor_tensor(out=shifted(t1b), in0=sm1s,
                                    in1=cbc(2, NBLK - 1), op=MULT)
            nc.vector.tensor_tensor(out=shifted(in2b), in0=shifted(in2b),
                                    in1=shifted(t1b), op=SUB)
            nc.vector.tensor_tensor_scan(
                out=csp[:], data0=d0_2d, data1=in1[:],
                initial=0.0, op0=MULT, op1=ADD)
            nc.vector.tensor_tensor(
                out=shifted(d2b),
                in0=_ap(csp[:], 0, [[NBLK, NBC], [1, NBLK - 1]]),
                in1=cbc(3, NBLK - 1), op=MULT)
            nc.vector.tensor_tensor(out=shifted(d2b), in0=shifted(d2b),
                                    in1=shifted(in2b), op=ADD)
            nc.scalar.copy(col0(d2b), cc16(10))
            nc.vector.tensor_tensor_scan(
                out=cep[:], data0=d0_2d, data1=d2[:],
                initial=0.0, op0=MULT, op1=ADD)
            # r1 = mu1*ce_prev + mu2*cs_prev ; r2 = nu*cs_prev
            nc.vector.tensor_tensor(out=t1[:], in0=ce_prev, in1=cbc(4), op=MULT)
            nc.vector.tensor_tensor(out=in1[:], in0=cs_prev, in1=cbc(5), op=MULT)
            nc.vector.tensor_tensor(out=_ap(rt[:], 0, [[2 * NBLK, NBC], [1, NBLK]]),
                                    in0=t1[:], in1=in1[:], op=ADD)
            nc.vector.tensor_tensor(out=_ap(rt[:], NBLK, [[2 * NBLK, NBC], [1, NBLK]]),
                                    in0=cs_prev, in1=cbc(6), op=MULT)

            for b_loc in range(HB):
                b = half * HB + b_loc
                r_ps = rps.tile([16, K], F32, tag="rps", name=f"rps{b}")
                for c in range(KC):
                    bc = b_loc * KC + c
                    nc.tensor.matmul(r_ps[:, c * P:(c + 1) * P],
                                     rt[:, bc, :, :], ident_t[:],
                                     is_transpose=True, skip_group_check=True)
                r_sb = rpool.tile([16, K], BF16, tag="rsb", name=f"rsb{b}")
                nc.scalar.copy(r_sb[:], r_ps[:])
                rsb[b] = r_sb

        def emit_C(b):
            """Main block matmuls, Act PSUM->SBUF copy, combine, DMA out."""
            v12 = vs.pop(b)
            r_sb = rsb.pop(b)
            out_stage = outpool.tile([P, NBLK, K], F32)
            yv = y[b].rearrange("(a p) k -> p a k", p=P)
            for j in range(NBLK // 2):
                pt = ppool.tile([P, 2, K], F32, tag="pt", name=f"pt{b}_{j}")
                stage = stagepool.tile([P, 2, K], F32, tag="stage",
                                       name=f"sg{b}_{j}")
                for half_i in (0, 1):
                    m = 2 * j + half_i
                    nc.tensor.matmul(pt[:, half_i, :], w_tril, v12[:, 0, m, :],
                                     start=True, stop=False,
                                     skip_group_check=True)
                    nc.tensor.matmul(pt[:, half_i, :], w_trilM, v12[:, 1, m, :],
                                     start=False, stop=False,
                                     skip_group_check=True)
                    nc.tensor.matmul(pt[:, half_i, :],
                                     wr_t[:, m * P:(m + 1) * P], r_sb[:],
                                     start=False, stop=True,
                                     skip_group_check=True)
                    nc.scalar.copy(stage[:, half_i, :], pt[:, half_i, :])
                eng = nc.vector if j % 2 == 0 else nc.gpsimd
                eng.tensor_tensor(out=out_stage[:, 2 * j:2 * j + 2, :],
                                  in0=stage[:], in1=e2_tile, op=MULT)
                nc.sync.dma_start(out=yv[:, 2 * j:2 * j + 2, :],
                                  in_=out_stage[:, 2 * j:2 * j + 2, :])

        # ---- software-pipelined emission -------------------------------
        # A0..A3 H0 C0 A4 C1 A5 C2 A6 C3 A7 H1 C4..C7
        for b in range(HB):
            emit_A(b)
        emit_H(0)
        emit_C(0)
        for b in range(1, HB):
            emit_A(HB + b - 1)
            emit_C(b)
        emit_A(2 * HB - 1)
        emit_H(1)
        for b in range(HB, 2 * HB):
            emit_C(b)

    nc.compile()
    return nc


_CACHE = {}
PROFILE = False
LAST_RESULT = None


def _host_constants(initial_level, tau):
    tau_c = np.maximum(tau.astype(np.float64), 1e-8)
    a = DT / tau_c
    e = np.exp(-a)
    em1 = 1.0 - e
    cEta = e * a * em1
    cS = em1 - e * a
    cEtp = a * em1  # cEta / e
    lvl = initial_level.astype(np.float64)

    i = np.arange(P, dtype=np.float64)[:, None]
    r = np.arange(P, dtype=np.float64)
    einv = np.exp(a[None, :] * i)               # e^{-i}
    G1 = cS[None, :] * einv
    G2 = cEtp[None, :] * einv
    E = np.exp(-a[None, :] * i)
    gte = np.concatenate([E, E], axis=1).astype(np.float32)
    g12 = np.concatenate([np.tile(G1, (1, 4)), np.tile(G2, (1, 4))],
                         axis=1).astype(np.float32)

    w_tril = (i <= r[None, :]).astype(np.float64)
    w_trilM = w_tril * (r[None, :] - i)
    wsum = np.zeros((P, P), dtype=np.float64)
    for m in range(NBLK):
        wsum[:, 16 * m + m] = 1.0
        wsum[:, 16 * m + 8 + m] = i[:, 0]
    wmat = np.concatenate([w_tril, w_trilM, wsum], axis=1).astype(
        ml_dtypes.bfloat16)

    wr_ = np.zeros((16, NBLK * P), dtype=np.float64)
    for m in range(NBLK):
        wr_[m, m * P:(m + 1) * P] = 1.0
        wr_[8 + m, m * P:(m + 1) * P] = r
    wr_ = wr_.astype(ml_dtypes.bfloat16)

    eL = e ** 128
    e127 = e ** 127
    e126 = e ** 126
    cs0 = lvl / em1
    ce0 = lvl / (em1 * em1)
    consts = [
        e127 / cEtp,           # 0 k1
        127.0 * e126 / cEtp,   # 1 k2
        e126 / cEtp,           # 2 k3
        128.0 * e127,          # 3 bL
        e * cEta,              # 4 mu1
        e * cS + cEta,         # 5 mu2
        cEta,                  # 6 nu
        eL * cs0,              # 7 alpha_e*cs0  (m=0 fixup)
        eL * ce0,              # 8 alpha_e*ce0  (m=0 fixup)
        cs0,                   # 9
        ce0,                   # 10
    ]
    chc = np.zeros((P, 11 * NBC + NBC * NBLK), dtype=np.float64)
    for bc in range(NBC):
        c = bc % KC
        sl = slice(c * P, (c + 1) * P)
        for j, v in enumerate(consts):
            chc[:, j * NBC + bc] = v[sl]
        # d0: scan multiplier alpha_e, 0 at m=0 (reset)
        for m in range(NBLK):
            chc[:, 11 * NBC + bc * NBLK + m] = 0.0 if m == 0 else eL[sl]
    chc = chc.astype(np.float32)

    ident = np.eye(P, dtype=np.float32)
    return gte, g12, wmat, wr_, ident, chc


def kernel(inputs, initial_level, tau):
    global LAST_RESULT
    inputs = np.ascontiguousarray(np.asarray(inputs, dtype=np.float32))
    initial_level = np.asarray(initial_level, dtype=np.float32)
    tau = np.asarray(tau, dtype=np.float32)
    assert inputs.shape == (B, T, K), inputs.shape

    gte, g12, wmat, wr_, ident, chc = _host_constants(initial_level, tau)

    if "nc" not in _CACHE:
        _CACHE["nc"] = build_nc()
    nc = _CACHE["nc"]

    in_maps = [
        {
            "x": inputs[i * B_LOC: (i + 1) * B_LOC],
            "gte": gte,
            "g12": g12,
            "wmat": wmat,
            "wr": wr_,
            "ident": ident,
            "chc": chc,
        }
        for i in range(N_CORES)
    ]
    res = run_bass_kernel_spmd(nc, in_maps, list(range(N_CORES)), trace=PROFILE)
    LAST_RESULT = res
    return np.concatenate([r["y"] for r in res.results], axis=0)
